# revision 11
# baseline (speedup 1.0000x reference)
"""Trainium2 Bass kernel for nn_BSHConv3D: spherical-harmonic 3^3 conv.

The whole module collapses to one dense 3D convolution
x[1,48,48,48,8] -> out[48,48,48, 512] with combined weights
W[3,3,3, 8, 512] (the central 1x1x1 conv folds into the center tap, the
bias rides on an extra constant-ones contraction row).

Per-core layout (D sharded 8 x 6 slabs, halo 1):
  - host pre-pads each core's x slab (plus halos) to [8ch + ones, 8*50*50]
  - on-chip: 9-way (kh,kw)-shifted im2col S[73, ~20k] built with
    SBUF->SBUF DMAs; kd handled as 3 PSUM-accumulating fp32r matmuls with
    free-dim offsets of +-2500
  - matmul: lhsT = S[:, zb+(kd-1)*2500 :][73 x 128], rhs = Wc[kd][73 x 512]
    -> PSUM [128 pos, 512 ch]
  - PSUM evacuated by VectorE/ScalarE alternating, valid rows DMA'd to HBM
"""

from contextlib import ExitStack

import numpy as np

import concourse.bass as bass
from concourse import bacc
import concourse.mybir as mybir
import concourse.tile as tile
from concourse.bass_utils import run_bass_kernel_spmd

B, D, H, W, C = 1, 48, 48, 48, 8
KS, R, DEG, NH, OUT = 3, 2, 3, 16, 16
NCORES = 8
DL = D // NCORES  # 6 output slabs per core
HP = WP = 50  # zero-padded H/W
SLAB = HP * WP  # 2500
NSLAB = DL + 2  # local slabs incl. halos
MARGIN = 64  # left margin in the z buffer (shift slack)
UD = NSLAB * SLAB  # 20000 payload columns
SZ = 20224  # total z columns per row
NCH = OUT * NH * 2  # 512 output channels (f, n, re/im)
KC = 73  # contraction rows: 9 taps x 8 ch + ones row
ZB0 = MARGIN + SLAB  # first computed z column
TM = 128  # positions per matmul tile
NT = 117  # z tiles per core
NVALID = DL * H * W  # 13824 valid output rows per core
NPAD = NT * TM  # 14976 padded output rows actually written
BUILD_LO = MARGIN
BUILD_HI = 20168  # build S over [BUILD_LO, BUILD_HI)
NZ_CHUNKS = 4  # im2col load chunking so matmuls can start early
GT = 9  # z tiles grouped per output DMA (117 = 13 groups of 9)

# module-level knobs for the test harness (graders just call kernel())
TRACE = False
LAST_RESULTS = None


def _valid_row_index():
    """Indices into the padded [NPAD] output rows that are real outputs,
    in output raster order."""
    u = np.arange(NPAD) + (ZB0 - MARGIN)
    dl = u // SLAB
    hp = (u % SLAB) // WP
    wp = u % WP
    mask = (dl >= 1) & (dl < 1 + DL) & (hp >= 1) & (hp <= H) & (wp >= 1) & (wp <= W)
    idx = np.nonzero(mask)[0]
    assert idx.size == NVALID, idx.size
    return idx


_VALID_IDX = _valid_row_index()


def _build_program():
    f32 = mybir.dt.float32
    f32r = mybir.dt.float32r
    nc = bacc.Bacc("TRN2", debug=False)
    xin = nc.dram_tensor("xin", [KC, SZ], f32r, kind="ExternalInput").ap()
    wc = nc.dram_tensor("wc", [3, KC, NCH], f32r, kind="ExternalInput").ap()
    out = nc.dram_tensor("out", [NPAD, NCH], f32, kind="ExternalOutput").ap()

    with tile.TileContext(nc) as tc, ExitStack() as ctx:
        const_pool = ctx.enter_context(tc.tile_pool(name="const", bufs=1))
        stage_pool = ctx.enter_context(tc.tile_pool(name="stage", bufs=2))
        psum_pool = ctx.enter_context(tc.tile_pool(name="psum", bufs=6, space="PSUM"))

        S = const_pool.tile([KC, SZ], f32r, name="S")
        Wt = const_pool.tile([KC, 3 * NCH], f32r, name="Wt")

        for kd in range(3):
            nc.sync.dma_start(Wt[:, kd * NCH : (kd + 1) * NCH], wc[kd])

        # im2col is pre-built host-side; just load it, chunked in z so the
        # first matmuls can start early
        nzc = (BUILD_HI - BUILD_LO + NZ_CHUNKS - 1) // NZ_CHUNKS
        for zc in range(NZ_CHUNKS):
            lo = BUILD_LO + zc * nzc
            hi = min(BUILD_HI, lo + nzc)
            nc.sync.dma_start(S[:, lo:hi], xin[:, lo:hi])

        for g0 in range(0, NT, GT):
            st = stage_pool.tile([TM, GT * NCH], f32, name="st")
            for g in range(GT):
                t = g0 + g
                zb = ZB0 + t * TM
                ps = psum_pool.tile([TM, NCH], f32, name="ps")
                for kd in range(3):
                    a = zb + (kd - 1) * SLAB
                    nc.tensor.matmul(
                        ps[:, :],
                        S[0:KC, a : a + TM],
                        Wt[0:KC, kd * NCH : (kd + 1) * NCH],
                        start=(kd == 0),
                        stop=(kd == 2),
                    )
                dst = st[:, g * NCH : (g + 1) * NCH]
                if t % 2 == 0:
                    nc.vector.tensor_copy(dst, ps[:, :])
                else:
                    nc.scalar.copy(dst, ps[:, :])
            # one DMA per group: SBUF [p, (g c)] -> DRAM rows [(g p), c],
            # iterated (p, g, c) so the SBUF partition dim stays dim 0
            src3 = st[:, :].rearrange("p (g c) -> p g c", g=GT)
            dst3 = out[g0 * TM : (g0 + GT) * TM, :].rearrange(
                "(g p) c -> p g c", p=TM
            )
            nc.sync.dma_start(dst3, src3)
    nc.compile()
    return nc


_program_cache = {}


def _get_program():
    if "nc" not in _program_cache:
        _program_cache["nc"] = _build_program()
    return _program_cache["nc"]


def _host_weights(atoms_real, atoms_imag, w, w_center, b_center):
    idx = np.repeat(np.arange(DEG + 1), [2 * n + 1 for n in range(DEG + 1)])
    w_exp = w[..., idx]  # [C,F,R,NH]
    WR = np.einsum("dhwrn,cfrn->dhwcfn", atoms_real, w_exp)
    WI = np.einsum("dhwrn,cfrn->dhwcfn", atoms_imag, w_exp)
    Wfull = np.stack([WR, WI], axis=-1)  # [3,3,3,C,F,NH,2]
    Wc = np.zeros((3, KC, NCH), np.float32)
    Wc[:, :72, :] = Wfull.reshape(3, 72, NCH)
    Wc[1, 32:40, 0::32] += w_center  # central 1x1x1 conv onto (f, n=0, re)
    Wc[1, 72, 0::32] = b_center
    return Wc


def kernel(x, atoms_real, atoms_imag, w, w_center, b_center):
    global LAST_RESULTS
    x = np.asarray(x, np.float32)
    Wc = _host_weights(
        np.asarray(atoms_real, np.float32),
        np.asarray(atoms_imag, np.float32),
        np.asarray(w, np.float32),
        np.asarray(w_center, np.float32),
        np.asarray(b_center, np.float32),
    )

    xt = np.transpose(x[0], (3, 0, 1, 2))  # [C,D,H,W]
    xpad = np.zeros((C, D + 2, HP, WP), np.float32)
    xpad[:, 1 : D + 1, 1 : H + 1, 1 : W + 1] = xt

    n_build = BUILD_HI - BUILD_LO
    in_maps = []
    for core in range(NCORES):
        d0 = core * DL
        pbuf = np.zeros((C, SZ), np.float32)
        pbuf[:, MARGIN : MARGIN + UD] = xpad[:, d0 : d0 + NSLAB].reshape(C, UD)
        buf = np.zeros((KC, SZ), np.float32)
        for kh in range(3):
            for kw in range(3):
                off = (kh - 1) * WP + (kw - 1)
                r0 = (kh * 3 + kw) * 8
                buf[r0 : r0 + 8, BUILD_LO:BUILD_HI] = pbuf[
                    :, BUILD_LO + off : BUILD_LO + off + n_build
                ]
        buf[72, :] = 1.0
        in_maps.append({"xin": buf, "wc": Wc})

    nc = _get_program()
    res = run_bass_kernel_spmd(
        nc, in_maps, core_ids=list(range(NCORES)), trace=TRACE
    )
    LAST_RESULTS = res
    outs = [res.results[i]["out"][_VALID_IDX] for i in range(NCORES)]
    full = np.concatenate([o.reshape(DL, H, W, OUT, NH, 2) for o in outs], axis=0)
    return full[None]


# revision 13
# speedup vs baseline: 1.0006x; 1.0006x over previous
"""Trainium2 Bass kernel for nn_BSHConv3D: spherical-harmonic 3^3 conv.

The whole module collapses to one dense 3D convolution
x[1,48,48,48,8] -> out[48,48,48, 512] with combined weights
W[3,3,3, 8, 512] (the central 1x1x1 conv folds into the center tap, the
bias rides on an extra constant-ones contraction row).

Per-core layout (D sharded 8 x 6 slabs, halo 1):
  - host pre-pads each core's x slab (plus halos) to [8ch + ones, 8*50*50]
  - on-chip: 9-way (kh,kw)-shifted im2col S[73, ~20k] built with
    SBUF->SBUF DMAs; kd handled as 3 PSUM-accumulating fp32r matmuls with
    free-dim offsets of +-2500
  - matmul: lhsT = S[:, zb+(kd-1)*2500 :][73 x 128], rhs = Wc[kd][73 x 512]
    -> PSUM [128 pos, 512 ch]
  - PSUM evacuated by VectorE/ScalarE alternating, valid rows DMA'd to HBM
"""

from contextlib import ExitStack

import numpy as np

import concourse.bass as bass
from concourse import bacc
import concourse.mybir as mybir
import concourse.tile as tile
from concourse.bass_utils import run_bass_kernel_spmd

B, D, H, W, C = 1, 48, 48, 48, 8
KS, R, DEG, NH, OUT = 3, 2, 3, 16, 16
NCORES = 8
DL = D // NCORES  # 6 output slabs per core
HP = WP = 50  # zero-padded H/W
SLAB = HP * WP  # 2500
NSLAB = DL + 2  # local slabs incl. halos
MARGIN = 64  # left margin in the z buffer (shift slack)
UD = NSLAB * SLAB  # 20000 payload columns
SZ = 20352  # total z columns per row
NCH = OUT * NH * 2  # 512 output channels (f, n, re/im)
KC = 73  # contraction rows: 9 taps x 8 ch + ones row
ZB0 = MARGIN + SLAB  # first computed z column
TM = 128  # positions per matmul tile
NT = 117  # z tiles per core
NVALID = DL * H * W  # 13824 valid output rows per core
NPAD = NT * TM  # 14976 padded output rows actually written
BUILD_LO = MARGIN
BUILD_HI = MARGIN + 20160  # build S over [BUILD_LO, BUILD_HI), 4*10*504 cols
NZ_CHUNKS = 4  # im2col load chunking so matmuls can start early
GT = 9  # z tiles grouped per output DMA (117 = 13 groups of 9)

# module-level knobs for the test harness (graders just call kernel())
TRACE = False
LAST_RESULTS = None


def _valid_row_index():
    """Indices into the padded [NPAD] output rows that are real outputs,
    in output raster order."""
    u = np.arange(NPAD) + (ZB0 - MARGIN)
    dl = u // SLAB
    hp = (u % SLAB) // WP
    wp = u % WP
    mask = (dl >= 1) & (dl < 1 + DL) & (hp >= 1) & (hp <= H) & (wp >= 1) & (wp <= W)
    idx = np.nonzero(mask)[0]
    assert idx.size == NVALID, idx.size
    return idx


_VALID_IDX = _valid_row_index()


def _build_program():
    f32 = mybir.dt.float32
    f32r = mybir.dt.float32r
    nc = bacc.Bacc("TRN2", debug=False)
    xin = nc.dram_tensor("xin", [KC, SZ], f32r, kind="ExternalInput").ap()
    wc = nc.dram_tensor("wc", [3, KC, NCH], f32r, kind="ExternalInput").ap()
    out = nc.dram_tensor("out", [NPAD, NCH], f32, kind="ExternalOutput").ap()

    with tile.TileContext(nc) as tc, ExitStack() as ctx:
        const_pool = ctx.enter_context(tc.tile_pool(name="const", bufs=1))
        stage_pool = ctx.enter_context(tc.tile_pool(name="stage", bufs=2))
        psum_pool = ctx.enter_context(tc.tile_pool(name="psum", bufs=6, space="PSUM"))

        S = const_pool.tile([KC, SZ], f32r, name="S")
        Wt = const_pool.tile([KC, 3 * NCH], f32r, name="Wt")

        for kd in range(3):
            nc.sync.dma_start(Wt[:, kd * NCH : (kd + 1) * NCH], wc[kd])

        # im2col is pre-built host-side; just load it, chunked in z so the
        # first matmuls can start early. The inner split keeps descriptors
        # at ~2KB so they spread across all 16 SDMA engines (a single big
        # per-row descriptor pins the whole load to one engine at ~26GB/s).
        nzc = (BUILD_HI - BUILD_LO) // NZ_CHUNKS
        zsub, csub = 10, nzc // 10
        assert zsub * csub == nzc and NZ_CHUNKS * nzc == BUILD_HI - BUILD_LO
        for zc in range(NZ_CHUNKS):
            lo = BUILD_LO + zc * nzc
            hi = lo + nzc
            src = xin[:, lo:hi].rearrange("p (zs c) -> p zs c", zs=zsub)
            dst = S[:, lo:hi].rearrange("p (zs c) -> p zs c", zs=zsub)
            nc.sync.dma_start(dst, src)

        for g0 in range(0, NT, GT):
            st = stage_pool.tile([TM, GT * NCH], f32, name="st")
            for g in range(GT):
                t = g0 + g
                zb = ZB0 + t * TM
                ps = psum_pool.tile([TM, NCH], f32, name="ps")
                for kd in range(3):
                    a = zb + (kd - 1) * SLAB
                    nc.tensor.matmul(
                        ps[:, :],
                        S[0:KC, a : a + TM],
                        Wt[0:KC, kd * NCH : (kd + 1) * NCH],
                        start=(kd == 0),
                        stop=(kd == 2),
                    )
                dst = st[:, g * NCH : (g + 1) * NCH]
                if t % 2 == 0:
                    nc.vector.tensor_copy(dst, ps[:, :])
                else:
                    nc.scalar.copy(dst, ps[:, :])
            # one DMA per group: SBUF [p, (g c)] -> DRAM rows [(g p), c],
            # iterated (p, g, c) so the SBUF partition dim stays dim 0
            src3 = st[:, :].rearrange("p (g c) -> p g c", g=GT)
            dst3 = out[g0 * TM : (g0 + GT) * TM, :].rearrange(
                "(g p) c -> p g c", p=TM
            )
            nc.sync.dma_start(dst3, src3)
    nc.compile()
    return nc


_program_cache = {}


def _get_program():
    if "nc" not in _program_cache:
        _program_cache["nc"] = _build_program()
    return _program_cache["nc"]


def _host_weights(atoms_real, atoms_imag, w, w_center, b_center):
    idx = np.repeat(np.arange(DEG + 1), [2 * n + 1 for n in range(DEG + 1)])
    w_exp = w[..., idx]  # [C,F,R,NH]
    WR = np.einsum("dhwrn,cfrn->dhwcfn", atoms_real, w_exp)
    WI = np.einsum("dhwrn,cfrn->dhwcfn", atoms_imag, w_exp)
    Wfull = np.stack([WR, WI], axis=-1)  # [3,3,3,C,F,NH,2]
    Wc = np.zeros((3, KC, NCH), np.float32)
    Wc[:, :72, :] = Wfull.reshape(3, 72, NCH)
    Wc[1, 32:40, 0::32] += w_center  # central 1x1x1 conv onto (f, n=0, re)
    Wc[1, 72, 0::32] = b_center
    return Wc


def kernel(x, atoms_real, atoms_imag, w, w_center, b_center):
    global LAST_RESULTS
    x = np.asarray(x, np.float32)
    Wc = _host_weights(
        np.asarray(atoms_real, np.float32),
        np.asarray(atoms_imag, np.float32),
        np.asarray(w, np.float32),
        np.asarray(w_center, np.float32),
        np.asarray(b_center, np.float32),
    )

    xt = np.transpose(x[0], (3, 0, 1, 2))  # [C,D,H,W]
    xpad = np.zeros((C, D + 2, HP, WP), np.float32)
    xpad[:, 1 : D + 1, 1 : H + 1, 1 : W + 1] = xt

    n_build = BUILD_HI - BUILD_LO
    in_maps = []
    for core in range(NCORES):
        d0 = core * DL
        pbuf = np.zeros((C, SZ), np.float32)
        pbuf[:, MARGIN : MARGIN + UD] = xpad[:, d0 : d0 + NSLAB].reshape(C, UD)
        buf = np.zeros((KC, SZ), np.float32)
        for kh in range(3):
            for kw in range(3):
                off = (kh - 1) * WP + (kw - 1)
                r0 = (kh * 3 + kw) * 8
                buf[r0 : r0 + 8, BUILD_LO:BUILD_HI] = pbuf[
                    :, BUILD_LO + off : BUILD_LO + off + n_build
                ]
        buf[72, :] = 1.0
        in_maps.append({"xin": buf, "wc": Wc})

    nc = _get_program()
    res = run_bass_kernel_spmd(
        nc, in_maps, core_ids=list(range(NCORES)), trace=TRACE
    )
    LAST_RESULTS = res
    outs = [res.results[i]["out"][_VALID_IDX] for i in range(NCORES)]
    full = np.concatenate([o.reshape(DL, H, W, OUT, NH, 2) for o in outs], axis=0)
    return full[None]
